# revision 9
# baseline (speedup 1.0000x reference)
"""Trainium2 Bass kernel for nn_LowRankLinear (y = x @ (U@V).T + bias).

Strategy:
  - Data-parallel: shard the 8192 tokens across 8 NeuronCores (1024 each).
  - Low-rank factorization on-device: t.T = (V @ x.T)  [rank x tok], then
    y.T = U @ t + bias — 34 GFLOP total instead of 283 GFLOP for the
    materialized-W reference.
  - All matmul operands are laid out on host so that the contraction dim is
    the partition dim (x.T, V.T, U.T) — every DMA is a natural strided load
    with fully contiguous per-partition lines, no on-device transposes. The
    output is produced as y.T (out_features on partitions) and transposed
    back on the host during the gather.
  - The kernel is DMA-bound (42 MB at ~360-425 GB/s/core). The design goal
    is one continuous DMA stream: V/U/bias interleaved into the x inflow
    (so matmuls never starve and U lands before matmul2), y.T outflow
    streaming immediately after at full DMA rate.
  - y.T orientation makes bias per-PARTITION, so each PSUM eviction is a
    single fused op alternating between DVE (tensor_scalar_add) and ACT
    (activation Identity+bias) — the two engines in parallel keep eviction
    off the critical path.
  - float32r matmuls (bit-identical to the fp32 2-pass PE path on TRN2,
    227 ns/matmul at N=512 steady-state, measured) with f32 PSUM.

Self-contained: hardcodes shapes from the problem spec; only needs the
concourse repo at /opt/trn_rl_repo (container-provided).
"""

import sys

if "/opt/trn_rl_repo" not in sys.path:
    sys.path.insert(0, "/opt/trn_rl_repo")

import numpy as np

import concourse.mybir as mybir
import concourse.tile as tile
from concourse import bacc
from concourse.bass_utils import run_bass_kernel_spmd

# Problem shapes (hardcoded per contract)
TOKENS = 8192
IN_F = 4096
OUT_F = 4096
RANK = 256
N_CORES = 8
TPC = TOKENS // N_CORES  # tokens per core = 1024

P = 128  # partitions
NG = 512  # moving free-dim per matmul (fp32 max, = 1 PSUM bank)
KC = IN_F // P  # 32 k-chunks for matmul1
RC = RANK // P  # 2 rank chunks
G = TPC // NG  # 2 halves of the token range (PSUM free-dim limit)
OFT = OUT_F // P  # 32 out_f tiles for matmul2
CB = 4  # k-chunks per x DMA (2 MB transfers, 4KB lines)
XD = KC // CB  # 8 x DMAs
VB = 8  # V.T k-chunks per DMA (1 MB)

F32 = mybir.dt.float32
MMDT = mybir.dt.float32r  # full-speed fp32 matmul mode

_CACHE = {}


def _build(mmdt):
    nc = bacc.Bacc(
        trn_type="TRN2", target_bir_lowering=False, debug=False, num_devices=N_CORES
    )
    xT = nc.dram_tensor("xT", [IN_F, TPC], mmdt, kind="ExternalInput")
    VT = nc.dram_tensor("VT", [IN_F, RANK], mmdt, kind="ExternalInput")
    UT = nc.dram_tensor("UT", [RANK, OUT_F], mmdt, kind="ExternalInput")
    # bias in column layout: bias_col[p, of] = bias[of*128 + p]
    biasc = nc.dram_tensor("biasc", [P, OFT], F32, kind="ExternalInput")
    yT = nc.dram_tensor("yT", [OUT_F, TPC], F32, kind="ExternalOutput")

    with tile.TileContext(nc) as tc:
        with (
            tc.tile_pool(name="const", bufs=1) as cp,
            tc.tile_pool(name="xp", bufs=3) as xp,
            tc.tile_pool(name="yp", bufs=4) as yp,
            tc.tile_pool(name="pt", bufs=4, space="PSUM") as ptp,
            tc.tile_pool(name="py", bufs=4, space="PSUM") as pyp,
        ):
            # ---- resident tensors ----
            vsb = cp.tile([P, KC * RANK], mmdt)  # V.T chunks [128, 256] x 32
            usb = cp.tile([P, RC * OUT_F], mmdt)  # U.T chunks [128, 4096] x 2
            tT = cp.tile([P, RC * TPC], mmdt)  # t.T  [rank-tile, tokens] x 2
            bcol = cp.tile([P, OFT], F32)  # per-partition bias columns

            def load_v(vd):
                nc.sync.dma_start(
                    vsb[:, vd * VB * RANK : (vd + 1) * VB * RANK].rearrange(
                        "p (c m) -> p c m", c=VB
                    ),
                    VT[vd * VB * P : (vd + 1) * VB * P, :].rearrange(
                        "(c p) m -> p c m", p=P
                    ),
                )

            def load_u(r):
                nc.sync.dma_start(
                    usb[:, r * OUT_F : (r + 1) * OUT_F], UT[r * P : (r + 1) * P, :]
                )

            x_tiles = {}

            def load_x(d):
                xt = xp.tile([P, CB, TPC], mmdt, name=f"xt{d}", tag="xt")
                nc.sync.dma_start(
                    xt[:],
                    xT[d * CB * P : (d + 1) * CB * P, :].rearrange(
                        "(c p) n -> p c n", p=P
                    ),
                )
                x_tiles[d] = xt

            # Single-ring (SP) inflow, interleaved so matmul1 never starves on
            # V, U lands before matmul2 starts, and x is continuous.
            load_v(0)
            load_x(0)
            load_v(1)
            load_x(1)
            load_v(2)
            load_x(2)
            load_v(3)
            load_x(3)
            load_u(0)
            load_x(4)
            load_u(1)
            load_x(5)
            nc.sync.dma_start(bcol[:], biasc[:])
            load_x(6)
            load_x(7)

            # ---- matmul1: t.T = sum_c V.T_c.T @ x.T_c over both token halves ----
            pt = [
                ptp.tile([P, NG], F32, name=f"pt{r}_{g}", tag="pt")
                for r in range(RC)
                for g in range(G)
            ]
            for d in range(XD):
                xt = x_tiles[d]
                for cc in range(CB):
                    c = d * CB + cc
                    for r in range(RC):
                        for g in range(G):
                            nc.tensor.matmul(
                                pt[r * G + g][:],
                                vsb[:, c * RANK + r * P : c * RANK + (r + 1) * P],
                                xt[:, cc, g * NG : (g + 1) * NG],
                                start=(c == 0),
                                stop=(c == KC - 1),
                            )
            for r in range(RC):
                for g in range(G):
                    # f32 PSUM -> f32r SBUF rounding copy
                    nc.vector.tensor_copy(
                        tT[:, r * TPC + g * NG : r * TPC + (g + 1) * NG],
                        pt[r * G + g][:],
                    )

            # ---- matmul2: y.T[of] = U.T_of.T @ t.T + bias ----
            # Eviction alternates DVE / ACT so both engines share the load;
            # stores are 512KB with 4KB contiguous lines on the SP ring.
            for of in range(OFT):
                ysb = yp.tile([P, TPC], F32)
                for g in range(G):
                    pyt = pyp.tile([P, NG], F32, tag="py")
                    for r in range(RC):
                        nc.tensor.matmul(
                            pyt[:],
                            usb[:, r * OUT_F + of * P : r * OUT_F + (of + 1) * P],
                            tT[:, r * TPC + g * NG : r * TPC + (g + 1) * NG],
                            start=(r == 0),
                            stop=(r == RC - 1),
                        )
                    if g == 0:
                        nc.vector.tensor_scalar_add(
                            ysb[:, g * NG : (g + 1) * NG],
                            pyt[:],
                            bcol[:, of : of + 1],
                        )
                    else:
                        nc.scalar.activation(
                            ysb[:, g * NG : (g + 1) * NG],
                            pyt[:],
                            mybir.ActivationFunctionType.Identity,
                            bias=bcol[:, of : of + 1],
                        )
                nc.sync.dma_start(yT[of * P : (of + 1) * P, :], ysb[:])
    nc.compile()
    return nc


def _get_nc():
    key = MMDT
    if key not in _CACHE:
        _CACHE[key] = _build(key)
    return _CACHE[key]


def _prep_in_maps(x, U, V, bias):
    x = np.ascontiguousarray(x, dtype=np.float32)
    VT = np.ascontiguousarray(np.asarray(V, dtype=np.float32).T)
    UT = np.ascontiguousarray(np.asarray(U, dtype=np.float32).T)
    bc = np.ascontiguousarray(
        np.asarray(bias, dtype=np.float32).reshape(OFT, P).T
    )
    in_maps = []
    for i in range(N_CORES):
        xTi = np.ascontiguousarray(x[i * TPC : (i + 1) * TPC, :].T)
        in_maps.append({"xT": xTi, "VT": VT, "UT": UT, "biasc": bc})
    return in_maps


def _gather(res):
    # res.results[i]["yT"] is [OUT_F, TPC]; full y is the token-major concat
    # of the transposes.
    yt = np.concatenate([res.results[i]["yT"] for i in range(N_CORES)], axis=1)
    return np.ascontiguousarray(yt.T)


def kernel(x, U, V, bias):
    nc = _get_nc()
    in_maps = _prep_in_maps(x, U, V, bias)
    res = run_bass_kernel_spmd(nc, in_maps, core_ids=list(range(N_CORES)))
    return _gather(res)


def run_profiled(x, U, V, bias, **trace_kwargs):
    """Like kernel() but with NTFF tracing; returns (y, BassKernelResults)."""
    nc = _get_nc()
    in_maps = _prep_in_maps(x, U, V, bias)
    res = run_bass_kernel_spmd(
        nc, in_maps, core_ids=list(range(N_CORES)), trace=True, **trace_kwargs
    )
    return _gather(res), res
